# revision 34
# baseline (speedup 1.0000x reference)
"""Sparse Conv3d (3x3x3 kmap) + BatchNorm + ReLU on 8 TRN2 NeuronCores. v3.

Design (voxel/data parallel, scatter-minimized):
  - Output voxels sharded across 8 cores (15000 rows each, padded to 15104).
  - Per-core output-row PERMUTATION (host-chosen): rows sorted by k1 = their
    first valid off-center offset, into fixed per-offset capacity blocks
    (cap1 = max count across cores, m=0 rows as filler), so the center
    contribution AND the first off-center contribution of every row are
    computed DIRECTLY into PSUM (accumulated center+pass1 matmuls per
    slot-pair, k-block pieces zero-padded on host) -- no scatter for them.
  - Only contributions 2..m of each row (~10.3K rows vs 21.9K) go through
    the SWDGE CCE scatter-add path: 4 parallel queues, FIXED accumulator
    pair per queue (no cross-queue WAW), negative-index padding (skipped by
    hardware, no trash packets), accumulators shrunk to 97 slots.
  - Engine spreading: memsets on vector/any/gpsimd, PSUM->SBUF copies split
    vector/scalar, merge chains split vector/any, normalize split, output
    DMA in two halves overlapped with normalize.
  - BN stats via PE ones-matmuls, AllReduce [1,128] across 8 cores,
    normalize + ReLU on-chip (bf16), row-wrapped bf16 output unwrapped +
    unpermuted on host.
"""

import sys
import os

for _p in ("/opt/trn_rl_repo", "/root/.axon_site/_ro/trn_rl_repo"):
    if os.path.isdir(_p) and _p not in sys.path:
        sys.path.insert(0, _p)

import numpy as np
import ml_dtypes

BF = ml_dtypes.bfloat16

N = 120000
CIN = 64
COUT = 64
K = 27
CENTER = 13
EPS = 1e-5
NCORES = 8
NC_ROWS = N // NCORES           # 15000
SLOTS = 118                     # ceil(15000/128)
WRAP_ROWS = SLOTS * 128         # 15104
HGRP = (SLOTS + 1) // 2         # 59 slot-pairs
NQ = 4                          # SWDGE queues
NOFS = K - 1                    # 26 off-center offsets


def _wrap16(idx):
    n = len(idx)
    assert n % 16 == 0
    w = np.ascontiguousarray(idx.reshape(n // 16, 16).T).astype(np.int16)
    return np.tile(w, (8, 1))


def _chpairs(rows):
    """[n*128, 64] rows -> chT pairs [128, n//2, 128] (bf16)."""
    n = rows.shape[0] // 128
    cht = rows.reshape(n, 128, CIN).transpose(0, 2, 1)      # [n, 64, 128]
    return np.ascontiguousarray(
        cht.reshape(n // 2, 2 * CIN, 128).transpose(1, 0, 2)).astype(BF)


def _plan(nbr):
    """Host index preprocessing.

    Returns static meta (shared across cores; defines the compiled
    structure) and per-core tensors."""
    offs = [k for k in range(K) if k != CENTER]

    per_core = []
    count1 = np.zeros((NCORES, NOFS), np.int64)
    for c in range(NCORES):
        seg = np.asarray(nbr[:, c * NC_ROWS:(c + 1) * NC_ROWS])
        valid = seg[offs] >= 0                  # [26, 15000]
        m = valid.sum(0)
        first = np.argmax(valid, axis=0)
        first = np.where(m > 0, first, -1)
        per_core.append((seg, valid, m, first))
        count1[c] = np.bincount(first[first >= 0], minlength=NOFS)
    cap1 = ((count1.max(0) + 1) // 2) * 2       # even per-offset capacity
    assert cap1.sum() + NTRASH <= NC_ROWS
    assert cap1.sum() + NTRASH <= MSLOT * 128   # scatter dsts fit acc slots
    base1 = NTRASH + np.concatenate([[0], np.cumsum(cap1)]).astype(np.int64)
    nd_rows = int(base1[-1])

    # direct-piece structure per slot-pair (compile-uniform)
    pieces_per_pair = []
    for p in range(HGRP):
        w0, w1 = 256 * p, 256 * p + 256
        pl = []
        for ki in range(NOFS):
            lo, hi = max(w0, int(base1[ki])), min(w1, int(base1[ki + 1]))
            if lo < hi:
                pl.append((ki, lo - w0, hi - w0))
            if int(base1[ki]) >= w1:
                break
        pieces_per_pair.append(pl)
    ndc = sum(len(pl) for pl in pieces_per_pair)

    # per-core permutation, direct srcs, scatter lists
    perms, d1s = [], []
    counts2 = np.zeros((NCORES, NOFS), np.int64)
    dsts2, srcs2 = {}, {}
    for c, (seg, valid, m, first) in enumerate(per_core):
        perm = np.full(WRAP_ROWS, -1, np.int64)   # region pos -> local row
        d1 = np.full(WRAP_ROWS, -1, np.int64)
        m0_rows = np.nonzero(m == 0)[0]
        fp = 0
        for ki in range(NOFS):
            rows_k = np.nonzero(first == ki)[0]
            nk = len(rows_k)
            lo = int(base1[ki])
            perm[lo:lo + nk] = rows_k
            d1[lo:lo + nk] = seg[offs[ki], rows_k]
            nfill = int(cap1[ki]) - nk
            perm[lo + nk:lo + nk + nfill] = m0_rows[fp:fp + nfill]
            fp += nfill
        rest = m0_rows[fp:]
        perm[nd_rows:nd_rows + len(rest)] = rest
        assert (perm >= 0).sum() == NC_ROWS
        perms.append(perm)
        d1s.append(d1)
        rpos = np.empty(NC_ROWS, np.int64)
        rpos[perm[perm >= 0]] = np.nonzero(perm >= 0)[0]
        for ki in range(NOFS):
            mask = valid[ki] & (first != ki)
            rows = np.nonzero(mask)[0]
            d = rpos[rows]
            order = np.argsort(d)
            dsts2[(c, ki)] = d[order]
            srcs2[(c, ki)] = seg[offs[ki], rows][order].astype(np.int64)
            counts2[c, ki] = len(rows)

    maxc = counts2.max(axis=0)
    maxc16 = ((maxc + 15) // 16) * 16                   # exact scatter extent
    chunks = ((maxc + 127) // 128).astype(np.int64)
    chunks = ((chunks + 1) // 2) * 2                    # even (pair chunks)

    # queue assignment: greedy balance by chunk count
    qload = [0] * NQ
    qoffs = [[] for _ in range(NQ)]
    for ki in np.argsort(-chunks):
        ki = int(ki)
        if chunks[ki] == 0:
            continue
        q = min(range(NQ), key=lambda x: qload[x])
        qoffs[q].append(ki)
        qload[q] += int(chunks[ki])
    qchunks = [int(l) for l in qload]
    for q in range(NQ):
        qoffs[q].sort(key=lambda ki: (int(chunks[ki]), ki))
    koff = {}
    for q in range(NQ):
        pos = 0
        for ki in qoffs[q]:
            koff[ki] = pos
            pos += int(chunks[ki])

    gsrc_cores, sidx_cores = [], []
    for c in range(NCORES):
        gq, sq = [], []
        for q in range(NQ):
            gstream = np.full(qchunks[q] * 128, -1, np.int64)
            # padding dsts cycle through the trash rows [0, NTRASH)
            sstream = np.arange(qchunks[q] * 128, dtype=np.int64) % NTRASH
            for ki in qoffs[q]:
                lo = koff[ki] * 128
                nv = int(counts2[c, ki])
                gstream[lo:lo + nv] = srcs2[(c, ki)]
                sstream[lo:lo + nv] = dsts2[(c, ki)]
            gq.append(gstream)
            sq.append(_wrap16(sstream))
        gsrc_cores.append(np.concatenate(gq))
        sidx_cores.append(np.concatenate(sq, axis=1))

    meta = dict(offs=offs, chunks=chunks, maxc16=maxc16, qoffs=qoffs,
                qchunks=qchunks, koff=koff, pieces=pieces_per_pair, ndc=ndc)
    return meta, perms, d1s, gsrc_cores, sidx_cores


# scatter dsts are < nd_rows <= MSLOT*128; accumulator slot counts
NTRASH = 32                     # trash rows [0, 32): scatter-padding targets
MSLOT = 98                      # slots reachable by scatter dsts
AE_G = 50                       # even slots 0,2,..,96 -> ceil(98/2)
AO_G = 50                       # matched to AE_G (scatter API needs equal shapes)


def _build_bass(meta):
    from concourse import mybir, bacc
    import concourse.tile as tile

    offs = meta["offs"]
    chunks = meta["chunks"]
    qoffs = meta["qoffs"]
    qchunks = meta["qchunks"]
    koff = meta["koff"]
    pieces = meta["pieces"]
    ndc = meta["ndc"]
    f32 = mybir.dt.float32
    bft = mybir.dt.bfloat16
    i16 = mybir.dt.int16
    gtot = sum(qchunks)
    qbase = np.cumsum([0] + qchunks)

    nc = bacc.Bacc("TRN2", target_bir_lowering=False, debug=False,
                   num_devices=NCORES, num_swdge_queues=NQ)
    ctrd = nc.dram_tensor("ctrd", [128, HGRP, 128], bft,
                          kind="ExternalInput").ap()
    drcd = nc.dram_tensor("drcd", [128, ndc, 128], bft,
                          kind="ExternalInput").ap()
    strd = nc.dram_tensor("strd", [128, gtot // 2, 128], bft,
                          kind="ExternalInput").ap()
    wmat = nc.dram_tensor("wmat", [128, K * 128], bft,
                          kind="ExternalInput").ap()
    sixd = nc.dram_tensor("sixd", [128, gtot * 8], i16,
                          kind="ExternalInput").ap()
    gbeta = nc.dram_tensor("gbeta", [1, 128], f32, kind="ExternalInput").ap()
    oute = nc.dram_tensor("oute", [128, HGRP, COUT], bft,
                          kind="ExternalOutput").ap()
    outo = nc.dram_tensor("outo", [128, HGRP, COUT], bft,
                          kind="ExternalOutput").ap()

    with tile.TileContext(nc) as tc:
        with tc.tile_pool(name="sb", bufs=1) as pool, \
             tc.tile_pool(name="ps", bufs=2, space="PSUM") as psum, \
             tc.tile_pool(name="dram", bufs=1, space="DRAM") as dram:
            # scatter indices FIRST on the Sync HWDGE ring (desc-gen dep)
            six = pool.tile([128, gtot * 8], i16)
            nc.sync.dma_start(out=six[:], in_=sixd[:])
            wsb = pool.tile([128, K * 128], bft)
            nc.sync.dma_start(out=wsb[:], in_=wmat[:])
            gb = pool.tile([1, 128], f32)
            nc.sync.dma_start(out=gb[:], in_=gbeta[:])
            # ---- accumulators (only slots reachable by scatter dsts) ----
            # memsets queue-ordered: q0/q1 pairs complete first on vector,
            # q2/q3 on the other engine, so early scatter calls unblock asap
            acc = []
            mseng = [nc.vector, nc.vector, nc.any, nc.any]
            for q in range(NQ):
                ae = pool.tile([128, AE_G, COUT], bft, tag=f"a{q}e")
                ao = pool.tile([128, AO_G, COUT], bft, tag=f"a{q}o")
                mseng[q].memset(ae[:], 0.0)
                mseng[q].memset(ao[:], 0.0)
                acc.append((ae, ao))
            me = pool.tile([128, HGRP, COUT], bft)   # merged/even slots
            mo = pool.tile([128, HGRP, COUT], bft)

            # bulk data: scatter streams on the Scalar ring, direct
            # chunks on the Sync ring (parallel), center last
            strm = pool.tile([128, gtot // 2, 128], bft)
            for q in range(NQ):
                lo, hi = int(qbase[q]) // 2, int(qbase[q + 1]) // 2
                if hi > lo:
                    nc.scalar.dma_start(out=strm[:, lo:hi, :],
                                        in_=strd[:, lo:hi, :])
            ctr = pool.tile([128, HGRP, 128], bft)
            nc.scalar.dma_start(out=ctr[:, 0:30, :], in_=ctrd[:, 0:30, :])
            nc.scalar.dma_start(out=ctr[:, 30:HGRP, :],
                                in_=ctrd[:, 30:HGRP, :])
            # direct chunks on the Sync ring: frees the Scalar HW queue
            # before the SWDGE scatter window (shared DMA engines)
            drc = pool.tile([128, ndc, 128], bft)
            nc.sync.dma_start(out=drc[:, 0:ndc // 2, :],
                              in_=drcd[:, 0:ndc // 2, :])
            nc.sync.dma_start(out=drc[:, ndc // 2:ndc, :],
                              in_=drcd[:, ndc // 2:ndc, :])

            onesb = pool.tile([128, 1], bft)
            nc.vector.memset(onesb[:], 1.0)
            onesr = pool.tile([1, 128], f32)
            nc.vector.memset(onesr[:], 1.0)

            # chunk -> piece-index table for direct matmuls
            piece_base = np.cumsum([0] + [len(pl) for pl in pieces])
            wc = wsb[:, CENTER * 128:(CENTER + 1) * 128]

            def emit_group(g0):
                gn = min(4, HGRP - g0)
                pyd = psum.tile([128, 8, COUT], f32, tag="pyd", bufs=2)
                for j in range(g0, g0 + gn):
                    t = (j - g0) * 2
                    pl = pieces[j]
                    nmm = 1 + len(pl)
                    nc.tensor.matmul(out=pyd[:, t:t + 2, :],
                                     lhsT=ctr[:, j, :], rhs=wc,
                                     start=True, stop=(nmm == 1))
                    for i, (ki, _, _) in enumerate(pl):
                        k = offs[ki]
                        dci = int(piece_base[j]) + i
                        nc.tensor.matmul(
                            out=pyd[:, t:t + 2, :],
                            lhsT=drc[:, dci, :],
                            rhs=wsb[:, k * 128:(k + 1) * 128],
                            start=False, stop=(i == len(pl) - 1))
                nc.vector.tensor_copy(out=me[:, g0:g0 + gn, :],
                                      in_=pyd[:, 0:2 * gn:2, :])
                nc.scalar.copy(out=mo[:, g0:g0 + gn, :],
                               in_=pyd[:, 1:2 * gn:2, :])

            group_list = list(range(0, HGRP, 4))
            gpos = 0

            # ---- scatter path: matmul -> copy -> CCE scatter-add ----
            ybfs = []
            for q in range(NQ):
                ybf_q = pool.tile([128, max(qchunks[q], 2), COUT], bft,
                                  tag=f"y{q}")
                ybfs.append(ybf_q)
            rounds = max(len(qoffs[q]) for q in range(NQ))
            for r in range(rounds):
                for q in range(NQ):
                    if r >= len(qoffs[q]):
                        continue
                    ki = qoffs[q][r]
                    k = offs[ki]
                    lo = koff[ki]
                    nidx = int(meta["maxc16"][ki])       # exact, 16-aligned
                    ncol = (nidx + 127) // 128
                    ncp = ((ncol + 1) // 2) * 2          # even (pair matmuls)
                    ybf = ybfs[q]
                    pbase = (int(qbase[q]) + lo) // 2
                    ceng = nc.vector if q % 2 == 0 else nc.scalar
                    for j0 in range(lo, lo + ncp, 8):
                        jn = min(8, lo + ncp - j0)       # even (2,4,6,8)
                        pys = psum.tile([128, 8, COUT], f32, tag="pys",
                                        bufs=3)
                        for p in range(jn // 2):
                            nc.tensor.matmul(
                                out=pys[:, 2 * p:2 * p + 2, :],
                                lhsT=strm[:, pbase + (j0 - lo) // 2 + p, :],
                                rhs=wsb[:, k * 128:(k + 1) * 128],
                                start=True, stop=True)
                        if q % 2 == 0:
                            ceng.tensor_copy(out=ybf[:, j0:j0 + jn, :],
                                             in_=pys[:, 0:jn, :])
                        else:
                            ceng.copy(out=ybf[:, j0:j0 + jn, :],
                                      in_=pys[:, 0:jn, :])
                    ae, ao = acc[q]
                    nidx = int(meta["maxc16"][ki])       # exact, 16-aligned
                    ncol = (nidx + 127) // 128
                    nc.gpsimd.dma_scatter_add(
                        out_ap=ae[:], in_ap=ybf[:, lo:lo + ncol, :],
                        idxs_ap=six[:, (int(qbase[q]) + lo) * 8:
                                    (int(qbase[q]) + lo) * 8 + nidx // 16],
                        num_idxs=nidx, num_idxs_reg=nidx,
                        elem_size=COUT, sbuf_tokens_per_rank=128,
                        parity_reg=0, out_ap_other=ao[:],
                        queue_num=q, single_packet=False)
            # direct/center groups AFTER all scatter matmuls: keeps the PE
            # queue from blocking early scatter rounds on the ctr/drc DMAs
            while gpos < len(group_list):
                emit_group(group_list[gpos])
                gpos += 1

            # ---- merge queue accumulators into me/mo (restricted) ----
            for q in range(NQ):
                ae, ao = acc[q]
                nc.vector.tensor_add(out=me[:, 0:AE_G, :],
                                     in0=me[:, 0:AE_G, :], in1=ae[:])
                nc.vector.tensor_add(out=mo[:, 0:AO_G, :],
                                     in0=mo[:, 0:AO_G, :], in1=ao[:])
            # trash rows [0, NTRASH) collected scatter padding: re-zero
            nc.vector.memset(me[0:NTRASH, 0:1, :], 0.0)

            # ---- stats: sums + sum-squares over all rows ----
            sqe2 = pool.tile([128, HGRP, COUT], bft, tag="sqe2")
            sqo2 = pool.tile([128, HGRP, COUT], bft, tag="sqo2")
            nc.vector.tensor_mul(out=sqe2[:], in0=me[:], in1=me[:])
            nc.any.tensor_mul(out=sqo2[:], in0=mo[:], in1=mo[:])
            psumr = psum.tile([1, 512], f32, tag="psumr", bufs=1)
            sum_ins = []
            for t in (me, mo):
                for g0 in range(0, HGRP, 8):
                    gn = min(8, HGRP - g0)
                    sum_ins.append(t[:, g0:g0 + gn, :])
            for i, ap in enumerate(sum_ins):
                w = ap.shape[1] * COUT
                nc.tensor.matmul(out=psumr[:, 0:w], lhsT=onesb[:], rhs=ap,
                                 start=(i == 0), stop=(i == len(sum_ins) - 1))
            psq = psum.tile([1, 512], f32, tag="pcov", bufs=1)
            sq_ins = []
            for t in (sqe2, sqo2):
                for g0 in range(0, HGRP, 8):
                    gn = min(8, HGRP - g0)
                    sq_ins.append(t[:, g0:g0 + gn, :])
            for i, ap in enumerate(sq_ins):
                w = ap.shape[1] * COUT
                nc.tensor.matmul(out=psq[:, 0:w], lhsT=onesb[:], rhs=ap,
                                 start=(i == 0), stop=(i == len(sq_ins) - 1))
            ssum = pool.tile([1, 512], f32)
            nc.vector.tensor_copy(out=ssum[:], in_=psumr[:])
            nc.vector.tensor_add(out=ssum[:, 0:256], in0=ssum[:, 0:256],
                                 in1=ssum[:, 256:512])
            nc.vector.tensor_add(out=ssum[:, 0:128], in0=ssum[:, 0:128],
                                 in1=ssum[:, 128:256])
            nc.vector.tensor_add(out=ssum[:, 0:64], in0=ssum[:, 0:64],
                                 in1=ssum[:, 64:128])
            qsum = pool.tile([1, 512], f32)
            nc.vector.tensor_copy(out=qsum[:], in_=psq[:])
            nc.vector.tensor_add(out=qsum[:, 0:256], in0=qsum[:, 0:256],
                                 in1=qsum[:, 256:512])
            nc.vector.tensor_add(out=qsum[:, 0:128], in0=qsum[:, 0:128],
                                 in1=qsum[:, 128:256])
            nc.vector.tensor_add(out=qsum[:, 0:64], in0=qsum[:, 0:64],
                                 in1=qsum[:, 64:128])
            stats = pool.tile([1, 128], f32)
            nc.vector.tensor_copy(out=stats[:, 0:64], in_=ssum[:, 0:64])
            nc.vector.tensor_copy(out=stats[:, 64:128], in_=qsum[:, 0:64])

            # ---- AllReduce over 8 cores ----
            cin_d = dram.tile([1, 128], f32)
            cout_d = dram.tile([1, 128], f32)
            nc.sync.dma_start(out=cin_d[:], in_=stats[:])
            nc.gpsimd.collective_compute(
                "AllReduce", mybir.AluOpType.add,
                replica_groups=[list(range(NCORES))],
                ins=[cin_d.opt()], outs=[cout_d.opt()])
            red = pool.tile([1, 128], f32)
            nc.sync.dma_start(out=red[:], in_=cout_d[:])

            # ---- affine params ----
            mean = pool.tile([1, COUT], f32)
            nc.vector.tensor_scalar_mul(out=mean[:], in0=red[:, 0:64],
                                        scalar1=1.0 / N)
            ex2 = pool.tile([1, COUT], f32)
            nc.vector.tensor_scalar_mul(out=ex2[:], in0=red[:, 64:128],
                                        scalar1=1.0 / N)
            var = pool.tile([1, COUT], f32)
            nc.vector.tensor_mul(out=var[:], in0=mean[:], in1=mean[:])
            nc.vector.tensor_sub(out=var[:], in0=ex2[:], in1=var[:])
            nc.vector.tensor_scalar_add(out=var[:], in0=var[:], scalar1=EPS)
            std = pool.tile([1, COUT], f32)
            nc.scalar.sqrt(out=std[:], in_=var[:])
            rstd = pool.tile([1, COUT], f32)
            nc.vector.reciprocal(out=rstd[:], in_=std[:])
            scl = pool.tile([1, COUT], f32)
            nc.vector.tensor_mul(out=scl[:], in0=gb[:, 0:64], in1=rstd[:])
            bia = pool.tile([1, COUT], f32)
            nc.vector.tensor_mul(out=bia[:], in0=mean[:], in1=scl[:])
            nc.vector.tensor_sub(out=bia[:], in0=gb[:, 64:128], in1=bia[:])

            # broadcast to [128, 8, 64] bf16
            pbs = psum.tile([128, COUT], f32, tag="pt", bufs=1)
            nc.tensor.matmul(out=pbs[:], lhsT=onesr[:], rhs=scl[:],
                             start=True, stop=True)
            s8 = pool.tile([128, 8, COUT], bft)
            nc.vector.tensor_copy(out=s8[:, 0, :], in_=pbs[:])
            pbb = psum.tile([128, COUT], f32, tag="pt", bufs=1)
            nc.tensor.matmul(out=pbb[:], lhsT=onesr[:], rhs=bia[:],
                             start=True, stop=True)
            b8 = pool.tile([128, 8, COUT], bft)
            nc.vector.tensor_copy(out=b8[:, 0, :], in_=pbb[:])
            for t8 in (s8, b8):
                nc.vector.tensor_copy(out=t8[:, 1:2, :], in_=t8[:, 0:1, :])
                nc.vector.tensor_copy(out=t8[:, 2:4, :], in_=t8[:, 0:2, :])
                nc.vector.tensor_copy(out=t8[:, 4:8, :], in_=t8[:, 0:4, :])

            # ---- normalize + relu in place (halves), overlap out DMA ----
            def norm_range(glo, ghi):
                for g0 in range(glo, ghi, 8):
                    gn = min(8, ghi - g0)
                    sl = me[:, g0:g0 + gn, :]
                    nc.vector.tensor_mul(out=sl, in0=sl, in1=s8[:, 0:gn, :])
                    nc.vector.tensor_add(out=sl, in0=sl, in1=b8[:, 0:gn, :])
                    nc.vector.tensor_scalar_max(out=sl, in0=sl, scalar1=0.0)
                    so = mo[:, g0:g0 + gn, :]
                    nc.any.tensor_mul(out=so, in0=so, in1=s8[:, 0:gn, :])
                    nc.any.tensor_add(out=so, in0=so, in1=b8[:, 0:gn, :])
                    nc.any.tensor_scalar_max(out=so, in0=so, scalar1=0.0)

            norm_range(0, 32)
            nc.sync.dma_start(out=oute[:, 0:32, :], in_=me[:, 0:32, :])
            nc.sync.dma_start(out=outo[:, 0:32, :], in_=mo[:, 0:32, :])
            norm_range(32, HGRP)
            nc.sync.dma_start(out=oute[:, 32:HGRP, :], in_=me[:, 32:HGRP, :])
            nc.sync.dma_start(out=outo[:, 32:HGRP, :], in_=mo[:, 32:HGRP, :])

    nc.compile()
    return nc


def _host_tensors(feats, weight, gamma, beta, meta, perms, d1s, gsrc_cores):
    feats = np.ascontiguousarray(np.asarray(feats, dtype=np.float32))
    weight = np.asarray(weight, dtype=np.float32)
    pieces = meta["pieces"]
    wm = np.zeros((128, K, 128), np.float32)
    for k in range(K):
        wm[0:64, k, 0:64] = weight[k]
        wm[64:128, k, 64:128] = weight[k]
    wm = np.ascontiguousarray(wm.reshape(128, K * 128)).astype(BF)
    gbv = np.zeros((1, 128), np.float32)
    gbv[0, 0:64] = np.asarray(gamma, np.float32)
    gbv[0, 64:128] = np.asarray(beta, np.float32)
    fz = np.concatenate([feats, np.zeros((1, CIN), np.float32)], axis=0)
    ndc = meta["ndc"]
    strms, ctrs, drcs = [], [], []
    for c in range(NCORES):
        gs = gsrc_cores[c]                    # -1 padding -> zero row
        strms.append(_chpairs(fz[gs]))
        # center stream: permuted rows (-1 region pos -> zero row)
        pm = perms[c]
        csr = np.where(pm >= 0, pm + c * NC_ROWS, len(feats)).astype(np.int64)
        ctrs.append(_chpairs(fz[csr]))
        # direct (pass-1) piece chunks
        D = fz[d1s[c]]                        # [15104, 64]
        dch = np.zeros((128, ndc, 128), BF)
        idx = 0
        for p, pl in enumerate(pieces):
            for (ki, rlo, rhi) in pl:
                win = np.zeros((256, CIN), np.float32)
                win[rlo:rhi] = D[256 * p + rlo:256 * p + rhi]
                dch[:, idx:idx + 1, :] = _chpairs(win)
                idx += 1
        drcs.append(dch)
    return wm, gbv, strms, ctrs, drcs


def _prepare_full(np_inputs):
    nbr = np.asarray(np_inputs["neighbor_idx"])
    meta, perms, d1s, gsrc_cores, sidx_cores = _plan(nbr)
    nc = _build_bass(meta)
    wm, gbv, strms, ctrs, drcs = _host_tensors(
        np_inputs["feats"], np_inputs["weight"], np_inputs["gamma"],
        np_inputs["beta"], meta, perms, d1s, gsrc_cores)
    in_maps = [
        {"strd": strms[c], "ctrd": ctrs[c], "drcd": drcs[c], "wmat": wm,
         "sixd": sidx_cores[c], "gbeta": gbv}
        for c in range(NCORES)
    ]
    return nc, in_maps, perms


def prepare(np_inputs):
    """Build the Bass module + per-core input maps (for test harnesses)."""
    nc, in_maps, _ = _prepare_full(np_inputs)
    return nc, in_maps


def kernel(feats, weight, gamma, beta, neighbor_idx):
    from concourse.bass_utils import run_bass_kernel_spmd

    np_inputs = {"feats": feats, "weight": weight, "gamma": gamma,
                 "beta": beta, "neighbor_idx": neighbor_idx}
    nc, in_maps, perms = _prepare_full(np_inputs)
    res = run_bass_kernel_spmd(nc, in_maps, core_ids=list(range(NCORES)))
    out = np.empty((N, COUT), np.float32)
    for c in range(NCORES):
        wrapped = np.empty((128, SLOTS, COUT), np.float32)
        wrapped[:, 0::2, :] = res.results[c]["oute"].astype(np.float32)
        wrapped[:, 1::2, :] = res.results[c]["outo"].astype(np.float32)
        rows = wrapped.transpose(1, 0, 2).reshape(WRAP_ROWS, COUT)
        pm = perms[c]
        sel = pm >= 0
        out[c * NC_ROWS + pm[sel]] = rows[sel]
    return out
